# revision 2
# baseline (speedup 1.0000x reference)
"""3x3 VALID conv (NCHW) on 8 Trainium2 NeuronCores, data-parallel on batch.

Contract: kernel(img, filtro) takes the FULL inputs
  img    [32, 128, 56, 56] f32
  filtro [256, 128, 3, 3]  f32
and returns the FULL output [32, 256, 54, 54] f32.

Strategy (per core, batch shard of 4 images):
- img lives in SBUF channels-on-partitions: [ci=128, n, h, w] (50KB/part).
  Image 0 is DMA'd in row chunks so the first matmul wave starts after
  ~19 rows land; images 1-3 stream in behind compute.
- filtro is host-transposed to [ci, (tap, co)] and replicated per core;
  each tap's [ci=128, co_block=128] slice is a matmul stationary operand.
- Output row-group of 9 rows x 54 cols = 486 pixels = one PSUM bank tile
  [co_block=128, 486]; 9 fp32r matmuls (one per 3x3 tap, K=ci=128)
  accumulate into it. The shifted rhs is a strided SBUF view (rows
  stride 56) - im2col without data movement. fp32r streams 1 col/cycle
  (4x faster than fp32 mode) at ~1.4e-4 scale-relative accuracy.
- Wave ordering: 4 row-groups in flight with taps outer, so consecutive
  matmuls reuse the same stationary weights; 8 PSUM banks double-buffer
  waves against the DVE drain.
- Output: per (n, co_block) a [128, 6*486] staging tile collects DVE
  psum drains; halves (0.75MB) are flushed to DRAM as they complete.
"""
from contextlib import ExitStack

import numpy as np

BATCH, C_IN, C_OUT, H, K = 32, 128, 256, 56, 3
OH = H - K + 1  # 54
N_CORES = 8
PER = BATCH // N_CORES  # 4
RG = 9          # output rows per matmul group; 9*54=486 <= 512-f32 PSUM bank
NG = OH // RG   # 6

_CACHE = {}


def _build(reps=1):
    import concourse.tile as tile
    from concourse import bacc, mybir

    FR = mybir.dt.float32r
    F32 = mybir.dt.float32
    wave_groups = 4
    first_chunks = 3
    out_split = 2

    nc = bacc.Bacc(None, target_bir_lowering=False)
    img = nc.declare_dram_parameter("img", [PER, C_IN, H, H], FR,
                                    isOutput=False)
    w = nc.declare_dram_parameter("w", [C_IN, K * K * C_OUT], FR,
                                  isOutput=False)
    out = nc.declare_dram_parameter("out", [PER, C_OUT, OH, OH], F32,
                                    isOutput=True)

    with tile.TileContext(nc) as tc:
        with ExitStack() as ctx:
            wpool = ctx.enter_context(tc.tile_pool(name="wpool", bufs=1))
            imgpool = ctx.enter_context(tc.tile_pool(name="imgpool", bufs=1))
            psum_pool = ctx.enter_context(
                tc.tile_pool(name="psum", bufs=8, space="PSUM"))
            outp = ctx.enter_context(tc.tile_pool(name="outp", bufs=4))

            def body():
                w_sb = wpool.tile([C_IN, K * K * C_OUT], FR)
                nc.sync.dma_start(out=w_sb, in_=w[:, :])
                img_sb = imgpool.tile([C_IN, PER, H, H], FR)
                src = img.rearrange("n c h w -> c n h w")
                bounds = [0]
                step = (H + first_chunks - 1) // first_chunks
                while bounds[-1] < H:
                    bounds.append(min(bounds[-1] + step, H))
                for r0, r1 in zip(bounds[:-1], bounds[1:]):
                    nc.sync.dma_start(out=img_sb[:, 0, r0:r1],
                                      in_=src[:, 0, r0:r1])
                for n in range(1, PER):
                    nc.sync.dma_start(out=img_sb[:, n], in_=src[:, n])

                for n in range(PER):
                    for cb in range(2):
                        stage = outp.tile([128, NG, RG * OH], F32,
                                          name="stage", tag="stage")
                        done_g = 0
                        for g0 in range(0, NG, wave_groups):
                            gs = range(g0, min(g0 + wave_groups, NG))
                            pss = {g: psum_pool.tile([128, RG * OH], F32,
                                                     name=f"ps{g}", tag="ps")
                                   for g in gs}
                            for t in range(K * K):
                                ki, kj = divmod(t, K)
                                col = t * C_OUT + cb * 128
                                lhsT = w_sb[:, col: col + 128]
                                for g in gs:
                                    rhs = img_sb[:, n,
                                                 g * RG + ki: g * RG + ki + RG,
                                                 kj: kj + OH]
                                    nc.tensor.matmul(
                                        pss[g], lhsT, rhs,
                                        start=(t == 0), stop=(t == K * K - 1))
                            for g in gs:
                                nc.vector.tensor_copy(stage[:, g], pss[g])
                            while (done_g < NG and
                                   (gs[-1] + 1 - done_g) >= NG // out_split):
                                lo, hi = done_g, done_g + NG // out_split
                                nc.sync.dma_start(
                                    out=out[n, cb * 128:(cb + 1) * 128,
                                            lo * RG: hi * RG],
                                    in_=stage[:, lo:hi].rearrange(
                                        "p g x -> p (g x)"))
                                done_g = hi

            if reps == 1:
                body()
            else:
                with tc.For_i(0, reps):
                    body()

    nc.finalize()
    return nc


def build(reps=1):
    return _build(reps=reps)


def kernel(img: np.ndarray, filtro: np.ndarray) -> np.ndarray:
    from concourse.bass_utils import run_bass_kernel_spmd

    img = np.ascontiguousarray(np.asarray(img, dtype=np.float32))
    filtro = np.asarray(filtro, dtype=np.float32)
    # w[ci, (ki*3+kj)*C_OUT + co] = filtro[co, ci, ki, kj]
    wt = np.ascontiguousarray(
        np.transpose(filtro, (1, 2, 3, 0))).reshape(C_IN, K * K * C_OUT)

    if "nc" not in _CACHE:
        _CACHE["nc"] = _build()
    nc = _CACHE["nc"]

    in_maps = [
        {"img": np.ascontiguousarray(img[c * PER:(c + 1) * PER]), "w": wt}
        for c in range(N_CORES)
    ]
    res = run_bass_kernel_spmd(nc, in_maps, list(range(N_CORES)))
    return np.concatenate(
        [res.results[c]["out"] for c in range(N_CORES)], axis=0)



# revision 5
# speedup vs baseline: 1.0927x; 1.0927x over previous
"""3x3 VALID conv (NCHW) on 8 Trainium2 NeuronCores, data-parallel on batch.

Contract: kernel(img, filtro) takes the FULL inputs
  img    [32, 128, 56, 56] f32
  filtro [256, 128, 3, 3]  f32
and returns the FULL output [32, 256, 54, 54] f32.

Strategy (per core, batch shard of 4 images):
- Inputs are host-cast to bf16 (rel-err budget 2e-2 >> bf16's ~4e-3 for
  K=1152 fp32-accumulated reductions). bf16 matmuls stream 1 col/cycle
  like fp32r, but their weight loads are emitted as standalone Ldweights
  instructions (FWL, ~53ns) that the PE's 64-deep reorder window hides
  behind in-flight matmuls - fp32r self-loading matmuls instead pay a
  serial ~107ns reload inside every matmul (~46us/rep).
- img in SBUF channels-on-partitions: [ci=128, n, h, w] bf16 (25KB/part).
- w host-packed to [ci, cb, tap, co128] so each (cb, tap) slice
  [128, 128] is one stationary load.
- Schedule: cb-major, then per image a wave of 6 row-groups (9 rows x 54
  = 486 cols = one PSUM bank each), taps outer so one weight load serves
  6 matmuls; 8-bank PSUM rotation lets the next wave start on banks 6,7
  while this wave's banks drain.
- Drain: DVE copies each bank to an SBUF tile; out DMA goes on the ACT
  HWDGE ring (nc.scalar.dma_start) so stores never head-of-line block
  the SP ring that prefetches the next iteration's images.
"""
from contextlib import ExitStack

import numpy as np

BATCH, C_IN, C_OUT, H, K = 32, 128, 256, 56, 3
OH = H - K + 1  # 54
N_CORES = 8
PER = BATCH // N_CORES  # 4
RG = 9          # output rows per matmul group; 9*54=486 <= 512-f32 PSUM bank
NG = OH // RG   # 6

_CACHE = {}


def _dedup_ldweights(nc):
    """Remove Ldweights whose stationary AP matches the weights already in
    the PE array (tile legalization emits one per matmul; taps-outer reuses
    each load 6x). Any waits/updates on a removed load move to the next PE
    instruction."""
    removed = 0
    for blk in nc.m.functions[0].blocks:
        insts = list(blk.instructions)
        last_w = None
        drop, pending = [], []
        for i, inst in enumerate(insts):
            if str(getattr(inst, "engine", "")) != "EngineType.PE":
                continue
            if pending and inst.opcode in ("Matmult", "Ldweights"):
                si = inst.sync_info
                waits = list(si.on_wait) if si else []
                ups = list(si.on_update) if si else []
                from concourse import mybir as _mb
                for psi in pending:
                    waits += list(psi.on_wait)
                    ups += list(psi.on_update)
                inst.sync_info = _mb.SyncInfo(on_wait=waits, on_update=ups)
                pending = []
            if inst.opcode == "Ldweights":
                sig = str(inst.ins[0])
                if sig == last_w:
                    si = inst.sync_info
                    if si and (len(si.on_wait) or len(si.on_update)):
                        pending.append(si)
                    drop.append(i)
                else:
                    last_w = sig
            elif inst.opcode == "Matmult":
                if inst.ldweights is not False:
                    last_w = str(inst.ins[1])
            else:
                last_w = None
        assert not pending
        for i in reversed(drop):
            del blk.instructions[i]
        removed += len(drop)
    return removed


def _build(reps=1):
    import concourse.tile as tile
    from concourse import bacc, mybir

    BF = mybir.dt.bfloat16
    F32 = mybir.dt.float32
    first_chunks = 3

    nc = bacc.Bacc(None, target_bir_lowering=False)
    img = nc.declare_dram_parameter("img", [PER, C_IN, H, H], BF,
                                    isOutput=False)
    w = nc.declare_dram_parameter("w", [C_IN, 2 * K * K * 128], BF,
                                  isOutput=False)
    out = nc.declare_dram_parameter("out", [PER, C_OUT, OH, OH], F32,
                                    isOutput=True)

    with tile.TileContext(nc) as tc:
        with ExitStack() as ctx:
            wpool = ctx.enter_context(tc.tile_pool(name="wpool", bufs=1))
            imgpool = ctx.enter_context(tc.tile_pool(name="imgpool", bufs=1))
            psum_pool = ctx.enter_context(
                tc.tile_pool(name="psum", bufs=8, space="PSUM"))
            outp = ctx.enter_context(tc.tile_pool(name="outp", bufs=8))

            def body():
                w_sb = wpool.tile([C_IN, 2 * K * K * 128], BF)
                # cb=0 block first so the first wave's weights land early
                half = K * K * 128
                nc.sync.dma_start(out=w_sb[:, :half], in_=w[:, :half])
                img_sb = imgpool.tile([C_IN, PER, H, H], BF)
                src = img.rearrange("n c h w -> c n h w")
                bounds = [0]
                step = (H + first_chunks - 1) // first_chunks
                while bounds[-1] < H:
                    bounds.append(min(bounds[-1] + step, H))
                for r0, r1 in zip(bounds[:-1], bounds[1:]):
                    nc.sync.dma_start(out=img_sb[:, 0, r0:r1],
                                      in_=src[:, 0, r0:r1])
                for n in range(1, PER):
                    nc.sync.dma_start(out=img_sb[:, n], in_=src[:, n])
                nc.sync.dma_start(out=w_sb[:, half:], in_=w[:, half:])

                for cb in range(2):
                    for n in range(PER):
                        pss = [psum_pool.tile([128, RG * OH], F32,
                                              name=f"ps{n}_{g}", tag="ps")
                               for g in range(NG)]
                        for t in range(K * K):
                            ki, kj = divmod(t, K)
                            col = (cb * K * K + t) * 128
                            lhsT = w_sb[:, col: col + 128]
                            for g in range(NG):
                                rhs = img_sb[:, n,
                                             g * RG + ki: g * RG + ki + RG,
                                             kj: kj + OH]
                                nc.tensor.matmul(
                                    pss[g], lhsT, rhs,
                                    start=(t == 0), stop=(t == K * K - 1))
                        for g in range(NG):
                            ob = outp.tile([128, RG * OH], F32,
                                           name="ob", tag="ob")
                            nc.vector.tensor_copy(ob, pss[g])
                            nc.scalar.dma_start(
                                out=out[n, cb * 128:(cb + 1) * 128,
                                        g * RG:(g + 1) * RG],
                                in_=ob.rearrange("p (r x) -> p r x", r=RG))

            if reps == 1:
                body()
            else:
                with tc.For_i(0, reps):
                    body()

    _dedup_ldweights(nc)
    nc.finalize()
    return nc


def build(reps=1):
    return _build(reps=reps)


def _prep(img: np.ndarray, filtro: np.ndarray):
    from ml_dtypes import bfloat16

    img_bf = np.ascontiguousarray(
        np.asarray(img, dtype=np.float32)).astype(bfloat16)
    filtro = np.asarray(filtro, dtype=np.float32)
    # w[ci, ((cb*9 + ki*3+kj)*128 + co128] = filtro[cb*128+co128, ci, ki, kj]
    wt = np.transpose(filtro, (1, 2, 3, 0)).reshape(C_IN, K, K, 2, 128)
    wt = np.ascontiguousarray(np.transpose(wt, (0, 3, 1, 2, 4))).reshape(
        C_IN, 2 * K * K * 128).astype(bfloat16)
    return img_bf, wt


def kernel(img: np.ndarray, filtro: np.ndarray) -> np.ndarray:
    from concourse.bass_utils import run_bass_kernel_spmd

    img_bf, wt = _prep(img, filtro)

    if "nc" not in _CACHE:
        _CACHE["nc"] = _build()
    nc = _CACHE["nc"]

    in_maps = [
        {"img": np.ascontiguousarray(img_bf[c * PER:(c + 1) * PER]),
         "w": wt}
        for c in range(N_CORES)
    ]
    res = run_bass_kernel_spmd(nc, in_maps, list(range(N_CORES)))
    return np.concatenate(
        [res.results[c]["out"] for c in range(N_CORES)], axis=0)


# revision 19
# speedup vs baseline: 1.2631x; 1.1560x over previous
"""3x3 VALID conv (NCHW) on 8 Trainium2 NeuronCores, data-parallel on batch.

Contract: kernel(img, filtro) takes the FULL inputs
  img    [32, 128, 56, 56] f32
  filtro [256, 128, 3, 3]  f32
and returns the FULL output [32, 256, 54, 54] f32.

Strategy (per core, batch shard of 4 images):
- Inputs are host-cast to bf16 (rel-err budget 2e-2 >> bf16's ~4e-3 for
  K=1152 fp32-accumulated reductions). bf16 matmuls stream 1 col/cycle
  like fp32r, but their weight loads are emitted as standalone Ldweights
  instructions (FWL, ~53ns) that the PE's 64-deep reorder window hides
  behind in-flight matmuls - fp32r self-loading matmuls instead pay a
  serial ~107ns reload inside every matmul (~46us/rep).
- img in SBUF channels-on-partitions: [ci=128, n, h, w] bf16 (25KB/part).
- w host-packed to [ci, cb, tap, co128] so each (cb, tap) slice
  [128, 128] is one stationary load.
- Schedule: cb-major, then per image a wave of 6 row-groups (9 rows x 54
  = 486 cols = one PSUM bank each), taps outer so one weight load serves
  6 matmuls; 8-bank PSUM rotation lets the next wave start on banks 6,7
  while this wave's banks drain.
- Drain: DVE copies each bank to an SBUF tile; out DMA goes on the ACT
  HWDGE ring (nc.scalar.dma_start) so stores never head-of-line block
  the SP ring that prefetches the next iteration's images.
"""
from contextlib import ExitStack

import numpy as np

BATCH, C_IN, C_OUT, H, K = 32, 128, 256, 56, 3
OH = H - K + 1  # 54
N_CORES = 8
PER = BATCH // N_CORES  # 4
RG = 9          # output rows per matmul group; 9*54=486 <= 512-f32 PSUM bank
NG = OH // RG   # 6

_CACHE = {}
DEDUP = True


def _dedup_ldweights(nc):
    """Remove Ldweights whose stationary AP matches the weights already in
    the PE array (tile legalization emits one per matmul; taps-outer reuses
    each load 6x). Any waits/updates on a removed load move to the next PE
    instruction."""
    removed = 0
    for blk in nc.m.functions[0].blocks:
        insts = list(blk.instructions)
        last_w = None
        drop, pending = [], []
        for i, inst in enumerate(insts):
            if str(getattr(inst, "engine", "")) != "EngineType.PE":
                continue
            if pending and inst.opcode in ("Matmult", "Ldweights"):
                si = inst.sync_info
                waits = list(si.on_wait) if si else []
                ups = list(si.on_update) if si else []
                from concourse import mybir as _mb
                for psi in pending:
                    waits += list(psi.on_wait)
                    ups += list(psi.on_update)
                inst.sync_info = _mb.SyncInfo(on_wait=waits, on_update=ups)
                pending = []
            if inst.opcode == "Ldweights":
                sig = str(inst.ins[0])
                if sig == last_w:
                    si = inst.sync_info
                    if si and (len(si.on_wait) or len(si.on_update)):
                        pending.append(si)
                    drop.append(i)
                else:
                    last_w = sig
            elif inst.opcode == "Matmult":
                if inst.ldweights is not False:
                    last_w = str(inst.ins[1])
            else:
                last_w = None
        assert not pending
        for i in reversed(drop):
            del blk.instructions[i]
        removed += len(drop)
    return removed


def _build(reps=1, drop_out=False, drop_mm=False, flat_rhs=False,
           single_w=False, drain_engine="vector", out_ring="scalar",
           wave_sz=4, in_bufs=2, mm_split=1):
    import concourse.tile as tile
    from concourse import bacc, mybir

    BF = mybir.dt.bfloat16
    F32 = mybir.dt.float32
    first_chunks = 3

    nc = bacc.Bacc(None, target_bir_lowering=False)
    img = nc.declare_dram_parameter("img", [PER, C_IN, H, H], BF,
                                    isOutput=False)
    w = nc.declare_dram_parameter("w", [C_IN, 2 * K * K * 128], BF,
                                  isOutput=False)
    out = nc.declare_dram_parameter("out", [PER, C_OUT, OH, OH], F32,
                                    isOutput=True)

    with tile.TileContext(nc) as tc:
        with ExitStack() as ctx:
            wpool = ctx.enter_context(
                tc.tile_pool(name="wpool", bufs=in_bufs))
            imgpool = ctx.enter_context(
                tc.tile_pool(name="imgpool", bufs=in_bufs))
            psum_pool = ctx.enter_context(
                tc.tile_pool(name="psum", bufs=8, space="PSUM"))
            outp = ctx.enter_context(tc.tile_pool(name="outp", bufs=8))

            def body():
                w_sb = wpool.tile([C_IN, 2 * K * K * 128], BF)
                # cb=0 block first so the first wave's weights land early
                half = K * K * 128
                nc.sync.dma_start(out=w_sb[:, :half], in_=w[:, :half])
                img_sb = imgpool.tile([C_IN, PER, H, H], BF)
                src = img.rearrange("n c h w -> c n h w")
                bounds = [0]
                step = (H + first_chunks - 1) // first_chunks
                while bounds[-1] < H:
                    bounds.append(min(bounds[-1] + step, H))
                for r0, r1 in zip(bounds[:-1], bounds[1:]):
                    nc.sync.dma_start(out=img_sb[:, 0, r0:r1],
                                      in_=src[:, 0, r0:r1])
                for n in range(1, PER):
                    nc.sync.dma_start(out=img_sb[:, n], in_=src[:, n])
                nc.sync.dma_start(out=w_sb[:, half:], in_=w[:, half:])

                groups = [(n, g) for n in range(PER) for g in range(NG)]
                for cb in range(2):
                    for w0 in range(0, len(groups), wave_sz):
                        wave = groups[w0:w0 + wave_sz]
                        pss = {ng: psum_pool.tile([128, RG * OH], F32,
                                                  name=f"ps{ng}", tag="ps")
                               for ng in wave}
                        if not drop_mm:
                            for t in range(K * K):
                                ki, kj = divmod(t, K)
                                col = (cb * K * K + t) * 128
                                if single_w:
                                    col = 0
                                lhsT = w_sb[:, col: col + 128]
                                for (n, g) in wave:
                                    if flat_rhs:
                                        rhs = img_sb[
                                            :, n].rearrange(
                                            "p h w -> p (h w)")[:, :RG * OH]
                                    else:
                                        rhs = img_sb[
                                            :, n,
                                            g * RG + ki: g * RG + ki + RG,
                                            kj: kj + OH]
                                    if mm_split == 1:
                                        nc.tensor.matmul(
                                            pss[(n, g)], lhsT, rhs,
                                            start=(t == 0),
                                            stop=(t == K * K - 1))
                                    else:
                                        assert flat_rhs
                                        cw = RG * OH // mm_split
                                        for s in range(mm_split):
                                            nc.tensor.matmul(
                                                pss[(n, g)][
                                                    :, s * cw:(s + 1) * cw],
                                                lhsT,
                                                rhs[:, s * cw:(s + 1) * cw],
                                                start=(t == 0),
                                                stop=(t == K * K - 1))
                        if drop_out or drop_mm:
                            continue
                        for (n, g) in wave:
                            ob = outp.tile([128, RG * OH], F32,
                                           name="ob", tag="ob")
                            if drain_engine == "vector":
                                nc.vector.tensor_copy(ob, pss[(n, g)])
                            else:
                                nc.scalar.copy(ob, pss[(n, g)])
                            dma_eng = (nc.scalar if out_ring == "scalar"
                                       else nc.sync)
                            dma_eng.dma_start(
                                out=out[n, cb * 128:(cb + 1) * 128,
                                        g * RG:(g + 1) * RG],
                                in_=ob.rearrange("p (r x) -> p r x", r=RG))

            if reps == 1:
                body()
            else:
                with tc.For_i(0, reps):
                    body()

    if DEDUP:
        _dedup_ldweights(nc)
    nc.finalize()
    return nc


def build(reps=1, **kw):
    return _build(reps=reps, **kw)


def _prep(img: np.ndarray, filtro: np.ndarray):
    from ml_dtypes import bfloat16

    img_bf = np.ascontiguousarray(
        np.asarray(img, dtype=np.float32)).astype(bfloat16)
    filtro = np.asarray(filtro, dtype=np.float32)
    # w[ci, ((cb*9 + ki*3+kj)*128 + co128] = filtro[cb*128+co128, ci, ki, kj]
    wt = np.transpose(filtro, (1, 2, 3, 0)).reshape(C_IN, K, K, 2, 128)
    wt = np.ascontiguousarray(np.transpose(wt, (0, 3, 1, 2, 4))).reshape(
        C_IN, 2 * K * K * 128).astype(bfloat16)
    return img_bf, wt


def kernel(img: np.ndarray, filtro: np.ndarray) -> np.ndarray:
    from concourse.bass_utils import run_bass_kernel_spmd

    img_bf, wt = _prep(img, filtro)

    if "nc" not in _CACHE:
        _CACHE["nc"] = _build()
    nc = _CACHE["nc"]

    in_maps = [
        {"img": np.ascontiguousarray(img_bf[c * PER:(c + 1) * PER]),
         "w": wt}
        for c in range(N_CORES)
    ]
    res = run_bass_kernel_spmd(nc, in_maps, list(range(N_CORES)))
    return np.concatenate(
        [res.results[c]["out"] for c in range(N_CORES)], axis=0)
